# revision 28
# baseline (speedup 1.0000x reference)
"""Block-causal sparse attention (MLA latent KV + GQA + RoPE) on 8 TRN2 cores.

Sharding: 2 batches x 4 query-chunks of 512 tokens (T-sharding). Each core
computes its 512 output rows end-to-end (q/kv projections, sparse attention,
o-projection) over a gathered key set of 768 window rows + 32 global slots.
Everything runs in a transposed layout (feature dim on partitions) so no
fp32 DMA-transpose is ever needed: the host supplies x^T slices and the
kernel returns out^T, which the host transposes back (free in numpy).

All matmul operands are bf16 (fp32 PSUM accumulation). Attention (stage C2)
computes S^T = K^T q directly (k on partitions) so the exp itself evacuates
PSUM->SBUF and no P transposes are needed; P@V runs with q on the output
partitions using an ones-augmented V column, so softmax row-sums appear as
output column 128 and the reciprocal is a cheap [128,1] DVE op. Only the
small y tiles (4 per head) are PE-transposed back for the o-projection.
"""

import functools
import numpy as np
import ml_dtypes

BF16 = ml_dtypes.bfloat16

# Model constants (hardcoded per problem spec)
D = 2048        # d_model
L = 512         # MLA latent
NH = 16         # query heads
NKV = 4         # kv heads
HD = 128        # head dim
B, T = 2, 2048
BLOCK = 128
WINDOW = 256
GEV = 64        # global every
THETA = 10000.0

# Sharding geometry
NCORES = 8
TQ = 512        # queries per core
KWIN = 768      # window key rows (t0-256 .. t0+512)
NG = 32         # global slots (padded)
KT = 896        # key layout: 768 window | 32 global | 96 zero pad
KQ0 = 256       # query cols inside key layout
NEG = -1e30
SCALE = 1.0 / float(np.sqrt(HD))

# S^T bank packing: (bank, column offset) of each window block w; widths are
# the number of valid q columns (q range [max(0,w-2)*128, (min(3,w)+1)*128)).
W_BANK = {0: (0, 0), 2: (0, 128), 3: (1, 0), 5: (1, 384), 1: (2, 0), 4: (2, 256)}
W_QLO = {w: max(0, w - 2) for w in range(6)}
W_QHI = {w: min(3, w) for w in range(6)}


def _build_program(loop_n=None):
    import contextlib
    import concourse.bacc as bacc
    import concourse.tile as tile
    import concourse.mybir as mybir

    f32 = mybir.dt.float32
    bf = mybir.dt.bfloat16
    EXP = mybir.ActivationFunctionType.Exp
    CPY = mybir.ActivationFunctionType.Copy

    nc = bacc.Bacc("TRN2", target_bir_lowering=False, debug=False)

    xT = nc.dram_tensor("xT", [D, KT], bf, kind="ExternalInput")
    wq = nc.dram_tensor("wq", [D, NH * HD], bf, kind="ExternalInput")
    wkv = nc.dram_tensor("wkv", [D, L], bf, kind="ExternalInput")
    wku = nc.dram_tensor("wku", [L, NKV * HD], bf, kind="ExternalInput")
    wvu = nc.dram_tensor("wvu", [L, NKV * HD], bf, kind="ExternalInput")
    wo = nc.dram_tensor("wo", [NH * HD, D], bf, kind="ExternalInput")
    cosT = nc.dram_tensor("cosT", [HD, KT], f32, kind="ExternalInput")
    sinT = nc.dram_tensor("sinT", [HD, KT], f32, kind="ExternalInput")
    # bias zone tables (transposed layout: [k, q]):
    #  tblA [128, 2, 128]: (w0,l0) | (w2,l2)   -> bankA cols 0:128, 384:512
    #  tblB [128, 128]:    (w3,l3)             -> bankB cols 256:384
    #  tblC [128, 256]:    (w1,l0|l1)          -> bankC cols 0:256
    #  tblG [32, 512]:     globals, all l      -> glob bank
    tblA = nc.dram_tensor("tblA", [128, 2, 128], bf, kind="ExternalInput")
    tblB = nc.dram_tensor("tblB", [128, 128], bf, kind="ExternalInput")
    tblC = nc.dram_tensor("tblC", [128, 256], bf, kind="ExternalInput")
    tblG = nc.dram_tensor("tblG", [32, 512], bf, kind="ExternalInput")
    rotM = nc.dram_tensor("rotM", [HD, HD], bf, kind="ExternalInput")
    ident = nc.dram_tensor("ident", [HD, HD], bf, kind="ExternalInput")
    outT = nc.dram_tensor("outT", [D, TQ], f32, kind="ExternalOutput")

    def act_copy(out, in_, scale=1.0):
        nc.scalar.activation(out, in_, CPY, scale=scale)

    with tile.TileContext(nc) as tc, contextlib.ExitStack() as _es:
        if loop_n:
            _es.enter_context(tc.For_i(0, loop_n, 1))
        with tc.tile_pool(name="const", bufs=1) as constp:
            cos_sb = constp.tile([HD, KT], f32)
            sin_sb = constp.tile([HD, KT], f32)
            tA_sb = constp.tile([128, 2, 128], bf)
            tB_sb = constp.tile([128, 128], bf)
            tC_sb = constp.tile([128, 256], bf)
            tG_sb = constp.tile([32, 512], bf)
            rot_sb = constp.tile([HD, HD], bf)
            id_sb = constp.tile([HD, HD], bf)

            def emit_const_dmas():
                # emitted after stage A's first x/w slices so the first
                # matmuls aren't stuck behind ~2MB of constant loads
                nc.sync.dma_start(cos_sb[:], cosT[:])
                nc.sync.dma_start(sin_sb[:], sinT[:])
                nc.sync.dma_start(tA_sb[:], tblA[:])
                nc.sync.dma_start(tB_sb[:], tblB[:])
                nc.sync.dma_start(tC_sb[:], tblC[:])
                nc.sync.dma_start(tG_sb[:], tblG[:])
                nc.sync.dma_start(rot_sb[:], rotM[:])
                nc.sync.dma_start(id_sb[:], ident[:])

            xtq_sb = constp.tile([128, 16, TQ], bf)      # x^T query cols
            qT_all = constp.tile([HD, NH, TQ], bf)       # roped q^T per head
            kT_sb = constp.tile([HD, NKV, KT], bf)       # roped K^T per kv head
            # V rows x (w, kv, hd + ones col). w=6 holds global-slot V rows.
            V_aug = constp.tile([128, 7, NKV, HD + 1], bf)
            nc.vector.memset(V_aug[:, :, :, HD:HD + 1], 1.0)
            yT_sb = constp.tile([HD, NH, TQ], bf)        # attention out^T
            y_all = constp.tile([128, NH, 4, HD], bf)    # y [q, hd] per (h, l)
            pD_sb = constp.tile([128, 16, TQ], f32)      # stage-D hk<8 partials

            with tc.tile_pool(name="ckvp", bufs=1) as ckvp:
                ckv_sb = ckvp.tile([128, 4, KT], bf)
                wku_sb = ckvp.tile([128, 4, NKV * HD], bf)
                wvu_sb = ckvp.tile([128, 4, NKV * HD], bf)

                def emit_kvw_dmas():
                    for lk in range(4):
                        nc.sync.dma_start(wku_sb[:, lk, :],
                                          wku[lk * 128:(lk + 1) * 128, :])
                        nc.sync.dma_start(wvu_sb[:, lk, :],
                                          wvu[lk * 128:(lk + 1) * 128, :])

                # ---- Stage A: c_kv^T = Wkv_down^T @ x^T  -> [L=4x128, KT] ----
                with tc.tile_pool(name="wx", bufs=3) as wxp, \
                     tc.tile_pool(name="psA", bufs=1, space="PSUM") as psA:
                    ps_ckv = [psA.tile([128, KT], f32, tag=f"ckv{lt}", name=f"ckv{lt}")
                              for lt in range(4)]
                    for k in range(16):
                        xt_k = wxp.tile([128, KT], bf, tag="xt")
                        nc.sync.dma_start(xt_k[:], xT[k * 128:(k + 1) * 128, :])
                        wkv_k = wxp.tile([128, L], bf, tag="wkv")
                        nc.sync.dma_start(wkv_k[:], wkv[k * 128:(k + 1) * 128, :])
                        if k == 0:
                            emit_const_dmas()
                            emit_kvw_dmas()
                        # stash query cols for stage C1
                        nc.vector.tensor_copy(xtq_sb[:, k, :], xt_k[:, KQ0:KQ0 + TQ])
                        for lt in range(4):
                            for c0, c1 in ((0, 512), (512, KT)):
                                nc.tensor.matmul(
                                    ps_ckv[lt][:, c0:c1],
                                    wkv_k[:, lt * 128:(lt + 1) * 128],
                                    xt_k[:, c0:c1],
                                    start=(k == 0), stop=(k == 15),
                                )
                    for lt in range(4):
                        nc.vector.tensor_copy(ckv_sb[:, lt, :], ps_ckv[lt][:])

                # ---- Stage C1: q projection + RoPE for all heads ----
                with tc.tile_pool(name="wqp", bufs=3) as wqp, \
                     tc.tile_pool(name="tmpQ", bufs=2) as tmpQ, \
                     tc.tile_pool(name="psq", bufs=2, space="PSUM") as psq, \
                     tc.tile_pool(name="psr", bufs=2, space="PSUM") as psr:
                    for h in range(NH):
                        wq_h = wqp.tile([128, 16, HD], bf, tag="wqh")
                        nc.sync.dma_start(
                            wq_h[:],
                            wq[:, h * HD:(h + 1) * HD].rearrange(
                                "(ko p) m -> p ko m", p=128),
                        )
                        ps_q = psq.tile([128, TQ], f32, tag="q")
                        for k in range(16):
                            nc.tensor.matmul(
                                ps_q[:], wq_h[:, k, :], xtq_sb[:, k, :],
                                start=(k == 0), stop=(k == 15),
                            )
                        qh_r = tmpQ.tile([128, TQ], bf, tag="qhr")
                        act_copy(qh_r[:], ps_q[:])
                        t_qc = tmpQ.tile([128, TQ], f32, tag="tqc")
                        nc.vector.tensor_mul(t_qc[:], ps_q[:],
                                             cos_sb[:, KQ0:KQ0 + TQ])
                        ps_rt = psr.tile([128, TQ], f32, tag="qrot")
                        nc.tensor.matmul(ps_rt[:], rot_sb[:], qh_r[:],
                                         start=True, stop=True)
                        qt1 = tmpQ.tile([128, TQ], f32, tag="qt1")
                        nc.vector.tensor_mul(qt1[:], ps_rt[:],
                                             sin_sb[:, KQ0:KQ0 + TQ])
                        nc.vector.tensor_add(qT_all[:, h, :], t_qc[:], qt1[:])

                # ---- Stage B: K^T (roped) and V ----
                # V-up matmul groups are interleaved between the K-rope
                # groups: the rope chain (PE->ACT->PE->DVE) leaves the PE
                # idle long enough for the HAM clock gate to re-throttle
                # right before C2; the dense V matmuls fill those gaps.
                with tc.tile_pool(name="tmpB", bufs=2) as tmpB, \
                     tc.tile_pool(name="psB", bufs=1, space="PSUM") as psB:
                    def emit_v(tt):
                        ps_v = psB.tile([128, 512], f32, tag="v")
                        for lk in range(4):
                            nc.tensor.matmul(
                                ps_v[:],
                                ckv_sb[:, lk, tt * 128:(tt + 1) * 128],
                                wvu_sb[:, lk, :],
                                start=(lk == 0), stop=(lk == 3),
                            )
                        nc.vector.tensor_copy(
                            V_aug[:, tt, :, 0:HD],
                            ps_v[:].rearrange("p (g d) -> p g d", g=NKV))

                    v_sched = {0: (0, 0), 1: (0, 1), 2: (1, 3), 3: (3, 7)}
                    for g in range(NKV):
                        ps_kh = psB.tile([128, KT], f32, tag="kh")
                        for lk in range(4):
                            for c0, c1 in ((0, 512), (512, KT)):
                                nc.tensor.matmul(
                                    ps_kh[:, c0:c1],
                                    wku_sb[:, lk, g * 128:(g + 1) * 128],
                                    ckv_sb[:, lk, c0:c1],
                                    start=(lk == 0), stop=(lk == 3),
                                )
                        kh_r = tmpB.tile([128, KT], bf, tag="khr")
                        act_copy(kh_r[:], ps_kh[:])
                        t_kc = tmpB.tile([128, KT], f32, tag="tkc")
                        nc.vector.tensor_mul(t_kc[:], ps_kh[:], cos_sb[:])
                        ps_rot = psB.tile([128, KT], f32, tag="rot")
                        for c0, c1 in ((0, 512), (512, KT)):
                            nc.tensor.matmul(ps_rot[:, c0:c1], rot_sb[:],
                                             kh_r[:, c0:c1], start=True, stop=True)
                        t1 = tmpB.tile([128, KT], f32, tag="t1")
                        nc.vector.tensor_mul(t1[:], ps_rot[:], sin_sb[:])
                        nc.vector.tensor_add(kT_sb[:, g, :], t_kc[:], t1[:])
                        for tt in range(*v_sched[g]):
                            emit_v(tt)

            # ---- Stage C2: sparse attention in S^T layout ----
            # S^T = K^T q (k rows on partitions) -> bias-add -> exp (evacuates
            # PSUM->SBUF) -> P@V with q on output partitions (ones column of
            # V_aug gives softmax sums in col 128) -> recip+scale -> transpose.
            with tc.tile_pool(name="ptp", bufs=2) as ptp, \
                 tc.tile_pool(name="rcp", bufs=2) as rcp, \
                 tc.tile_pool(name="wo1p", bufs=2) as wo1p, \
                 tc.tile_pool(name="psS", bufs=1, space="PSUM") as psS, \
                 tc.tile_pool(name="psG", bufs=1, space="PSUM") as psG, \
                 tc.tile_pool(name="psY", bufs=1, space="PSUM") as psY, \
                 tc.tile_pool(name="psD1", bufs=1, space="PSUM") as psD1, \
                 tc.tile_pool(name="psT", bufs=1, space="PSUM") as psT:

                st = psS.tile([128, 3, 512], f32, name="st")
                stg = psG.tile([32, 512], f32, name="stg")
                ya = psY.tile([128, 2, HD + 1], f32, name="ya")
                yb = psY.tile([128, 2, HD + 1], f32, name="yb")
                pt_tiles = [None] * NH
                r_tiles = [None] * NH

                def st_pe_ops(h):
                    """PE ops of head h's S^T, with bias/exp emitted inline as
                    their input banks complete (w order: banks A, B, C, glob)."""
                    g = h // 4
                    ptw = ptp.tile([128, 3, 512], bf, tag="ptw",
                                   name=f"ptw{h % 2}")
                    ptg = ptp.tile([32, 512], bf, tag="ptg", name=f"ptg{h % 2}")
                    pt_tiles[h] = (ptw, ptg)

                    def mk_w(w, start=False):
                        def op():
                            b, off = W_BANK[w]
                            qlo, qhi = W_QLO[w], W_QHI[w]
                            nc.tensor.matmul(
                                st[:, b, off:off + (qhi - qlo + 1) * 128],
                                kT_sb[:, g, w * 128:(w + 1) * 128],
                                qT_all[:, h, qlo * 128:(qhi + 1) * 128],
                                start=start, stop=True,
                                skip_group_check=True,
                            )
                        return op

                    # Bias zones are seeded by identity-weight matmuls
                    # (start=True writes the bias table into PSUM and sets
                    # has_written only in the bias zones; the S^T matmuls then
                    # accumulate onto them, overwriting elsewhere). This keeps
                    # the per-head critical cycle st(h+1) -> exp(h) -> st(h)
                    # on two engines instead of three (no DVE hop).
                    # Bank C (w1, w4) is built and exp'd first so head h+1's
                    # first matmul only waits on this early exp.
                    def bias_mm(out_ap, tbl_ap, lhs, start=True):
                        # start=True on the first bias matmul of a bank clears
                        # the bank's stale has_written bits from the previous
                        # head; later seeds in the same bank use start=False
                        # (bits there are clear, so they overwrite).
                        nc.tensor.matmul(out_ap, lhs, tbl_ap,
                                         start=start, stop=True,
                                         skip_group_check=True)

                    def op_biases():
                        # all bias seeds back to back: one id stationary load
                        # serves four matmuls (same-weight streams pipeline at
                        # N cycles instead of paying fill+drain each)
                        bias_mm(st[:, 2, 0:256], tC_sb[:], id_sb[:])
                        bias_mm(st[:, 0, 0:128], tA_sb[:, 0, :], id_sb[:])
                        bias_mm(st[:, 0, 384:512], tA_sb[:, 1, :], id_sb[:],
                                start=False)
                        bias_mm(st[:, 1, 256:384], tB_sb[:], id_sb[:])
                        bias_mm(stg[:], tG_sb[:], id_sb[0:NG, 0:NG])

                    def op_bankC():
                        mk_w(4)()
                        nc.scalar.activation(ptw[:, 2, :], st[:, 2, :], EXP,
                                             scale=SCALE)

                    def op_bankB():
                        mk_w(5)()
                        nc.scalar.activation(ptw[:, 0:2, :], st[:, 0:2, :], EXP,
                                             scale=SCALE)

                    def op_glob():
                        nc.tensor.matmul(
                            stg[:], kT_sb[:, g, KWIN:KWIN + NG],
                            qT_all[:, h, :], start=False, stop=True,
                            skip_group_check=True,
                        )
                        nc.scalar.activation(ptg[:], stg[:], EXP, scale=SCALE)

                    return [op_biases, mk_w(1), op_bankC, mk_w(0),
                            mk_w(2), mk_w(3), op_bankB, op_glob]

                def pv_pe_ops(h):
                    """16 PV matmuls of head h (each one LDWEIGHTS + short MM;
                    interleaved with the next head's S^T streams to keep the
                    PE array busy so HAM stays warm)."""
                    g = h // 4
                    ptw, ptg = pt_tiles[h]
                    ops = []
                    for l in range(4):
                        yt = (ya, yb)[l // 2]
                        for i, w in enumerate((l, l + 1, l + 2)):
                            def op(l=l, w=w, i=i, yt=yt):
                                b, off = W_BANK[w]
                                zoff = off + (l - W_QLO[w]) * 128
                                nc.tensor.matmul(
                                    yt[:, l % 2, :],
                                    ptw[:, b, zoff:zoff + 128],
                                    V_aug[:, w, g, :],
                                    start=(i == 0), stop=False,
                                    skip_group_check=True,
                                )
                            ops.append(op)

                        def opg(l=l, yt=yt):
                            nc.tensor.matmul(
                                yt[:, l % 2, :],
                                ptg[:, l * 128:(l + 1) * 128],
                                V_aug[0:NG, 6, g, :],
                                start=False, stop=True, skip_group_check=True,
                            )
                        ops.append(opg)
                    return ops

                def pv_post(h):
                    ra = rcp.tile([128, 2, 1], f32, tag="ra", name=f"ra{h % 2}")
                    rb = rcp.tile([128, 2, 1], f32, tag="rb", name=f"rb{h % 2}")
                    r_tiles[h] = (ra, rb)
                    nc.vector.reciprocal(ra[:], ya[:, :, HD:HD + 1])
                    nc.vector.reciprocal(rb[:], yb[:, :, HD:HD + 1])
                    # normalize + evacuate y [q, hd] (alternate DVE/ACT)
                    for l in range(4):
                        yt = (ya, yb)[l // 2]
                        rt = (ra, rb)[l // 2]
                        if l == 1:
                            nc.scalar.activation(
                                y_all[:, h, l, :], yt[:, l % 2, 0:HD], CPY,
                                scale=rt[:, l % 2, :])
                        else:
                            nc.vector.tensor_scalar_mul(
                                y_all[:, h, l, :], yt[:, l % 2, 0:HD],
                                rt[:, l % 2, :])

                def transp(h):
                    ps_t = psT.tile([128, 512], bf, tag="pt", name="pst")
                    for l in range(4):
                        nc.tensor.transpose(
                            ps_t[:, l * 128:(l + 1) * 128], y_all[:, h, l, :],
                            id_sb[:])
                    if h % 2 == 0:
                        nc.vector.tensor_copy(yT_sb[:, h, :], ps_t[:])
                    else:
                        act_copy(yT_sb[:, h, :], ps_t[:])

                def d1_od(od):
                    # first half of the o-projection (hk 0..7), interleaved
                    # into the back half of the head loop: its long N=512
                    # streams fill C2's dependency stalls and keep the HAM
                    # clock gate warm; partials land in SBUF for stage D2.
                    wo_od = wo1p.tile([128, 8, 128], bf, tag="wo1")
                    nc.sync.dma_start(
                        wo_od[:],
                        wo[0:1024, od * 128:(od + 1) * 128].rearrange(
                            "(ko p) m -> p ko m", p=128),
                    )
                    ps_o = psD1.tile([128, TQ], f32, tag="d1", name="d1")
                    for hk in range(8):
                        nc.tensor.matmul(
                            ps_o[:], wo_od[:, hk, :], yT_sb[:, hk, :],
                            start=(hk == 0), stop=(hk == 7),
                        )
                    nc.vector.tensor_copy(pD_sb[:, od, :], ps_o[:])

                def interleave(st_ops, pv_ops):
                    # ~2 PV matmuls (LDW-heavy, short streams) per S^T op
                    # (long streams): the S^T streaming hides the PV weight
                    # loads so the PE array never idles long enough to cool.
                    si, pi = 0, 0
                    while si < len(st_ops) or pi < len(pv_ops):
                        if si < len(st_ops):
                            st_ops[si]()
                            si += 1
                        for _ in range(3):
                            if pi < len(pv_ops):
                                pv_ops[pi]()
                                pi += 1

                d1_sched = {8: (0, 2), 9: (2, 4), 10: (4, 6), 11: (6, 8),
                            12: (8, 10), 13: (10, 12), 14: (12, 14),
                            15: (14, 16)}
                for h in range(NH):
                    interleave(st_pe_ops(h), pv_pe_ops(h - 1) if h else [])
                    if h > 0:
                        pv_post(h - 1)
                        transp(h - 1)
                    for od in range(*d1_sched.get(h, (0, 0))):
                        d1_od(od)
                interleave([], pv_pe_ops(NH - 1))
                pv_post(NH - 1)
                transp(NH - 1)

            # ---- Stage D2: second half of o-projection (hk 8..15) + partials
            with tc.tile_pool(name="wop", bufs=3) as wop, \
                 tc.tile_pool(name="tmpD", bufs=3) as tmpD, \
                 tc.tile_pool(name="psD", bufs=2, space="PSUM") as psD:
                for od in range(16):
                    wo_od = wop.tile([128, 8, 128], bf, tag="wo")
                    nc.sync.dma_start(
                        wo_od[:],
                        wo[1024:2048, od * 128:(od + 1) * 128].rearrange(
                            "(ko p) m -> p ko m", p=128),
                    )
                    ps_o = psD.tile([128, TQ], f32, tag="o")
                    for hk in range(8):
                        nc.tensor.matmul(
                            ps_o[:], wo_od[:, hk, :], yT_sb[:, 8 + hk, :],
                            start=(hk == 0), stop=(hk == 7),
                        )
                    ob = tmpD.tile([128, TQ], f32, tag="ob")
                    nc.vector.tensor_add(ob[:], ps_o[:], pD_sb[:, od, :])
                    nc.sync.dma_start(outT[od * 128:(od + 1) * 128, :], ob[:])

    nc.finalize()
    return nc


@functools.lru_cache(maxsize=1)
def _program():
    return _build_program()


def _rope_tables():
    freqs = 1.0 / (THETA ** (np.arange(0, HD, 2, dtype=np.float32) / HD))
    emb = np.arange(T, dtype=np.float32)[:, None] * freqs[None, :]  # [T, 64]
    cos = np.concatenate([np.cos(emb), np.cos(emb)], axis=-1)  # [T, 128]
    sin = np.concatenate([np.sin(emb), np.sin(emb)], axis=-1)
    return cos.astype(np.float32), sin.astype(np.float32)


def _masked(qpos, kpos):
    """Reference sparsity rule. qpos [Q], kpos [K] -> bool [Q, K] (True=masked)."""
    qb = qpos[:, None] // BLOCK
    kb = kpos[None, :] // BLOCK
    future = kb > qb
    outside = np.abs(kpos[None, :] - qpos[:, None]) > WINDOW
    glob = (kpos[None, :] % GEV) == 0
    return (outside & ~glob) | future


def _zone_bias(kp, t0, w, l):
    """Bias [128 k, 128 q] for window block w vs query block l (S^T layout)."""
    kpos = kp[w * 128:(w + 1) * 128]
    qpos = t0 + l * 128 + np.arange(128)
    m = np.where(kpos[:, None] < 0, True,
                 _masked(qpos, np.maximum(kpos, 0)).T)
    return np.where(m, NEG, 0.0).astype(np.float32)


def _core_inputs(x, Wq, Wkv, Wku, Wvu, Wo, cos, sin, b, ch):
    t0 = ch * TQ
    kp = np.full(KT, -1, dtype=np.int64)
    kp[0:KWIN] = np.arange(t0 - WINDOW, t0 + TQ)
    globpos = np.arange(0, max(0, t0 - WINDOW + 3 * 128), GEV)
    assert len(globpos) <= NG
    kp[KWIN:KWIN + len(globpos)] = globpos
    valid = kp >= 0

    xT = np.zeros((D, KT), BF16)
    xT[:, valid] = x[b, kp[valid]].T.astype(BF16)
    cosT = np.zeros((HD, KT), np.float32)
    sinT = np.zeros((HD, KT), np.float32)
    cosT[:, valid] = cos[kp[valid]].T
    sinT[:, valid] = sin[kp[valid]].T

    # Window-zone bias tables (S^T layout: [k rows, q cols]).
    tblA = np.stack([_zone_bias(kp, t0, 0, 0), _zone_bias(kp, t0, 2, 2)], 0)
    tblA = tblA.transpose(1, 0, 2)                     # [128, 2, 128]
    tblB = _zone_bias(kp, t0, 3, 3)                    # [128, 128]
    tblC = np.concatenate([_zone_bias(kp, t0, 1, 0),
                           _zone_bias(kp, t0, 1, 1)], axis=1)  # [128, 256]
    # sanity: every other (w, l) zone must be mask-free
    for w in range(6):
        for l in range(W_QLO[w], W_QHI[w] + 1):
            if (w, l) in ((0, 0), (2, 2), (3, 3), (1, 0), (1, 1)):
                continue
            assert not (_zone_bias(kp, t0, w, l) < 0).any(), (w, l)

    # Globals bias [32 slots, 512 q]: validity + sparsity + in-window dedup.
    kg = kp[KWIN:KWIN + NG]
    qpos = t0 + np.arange(TQ)
    mg = np.where(kg[:, None] < 0, True,
                  _masked(qpos, np.maximum(kg, 0)).T)
    lblk = np.arange(TQ) // 128
    mg = mg | (kg[:, None] >= (t0 - WINDOW + lblk[None, :] * 128))
    tblG = np.where(mg, NEG, 0.0).astype(np.float32)

    rotM = np.zeros((HD, HD), np.float32)
    rotM[np.arange(64), np.arange(64) + 64] = 1.0
    rotM[np.arange(64) + 64, np.arange(64)] = -1.0
    ident = np.eye(HD, dtype=np.float32)

    return dict(xT=xT,
                wq=Wq.astype(BF16), wkv=Wkv.astype(BF16),
                wku=Wku.astype(BF16), wvu=Wvu.astype(BF16),
                wo=Wo.astype(BF16),
                cosT=cosT, sinT=sinT,
                tblA=tblA.astype(BF16), tblB=tblB.astype(BF16),
                tblC=tblC.astype(BF16), tblG=tblG.astype(BF16),
                rotM=rotM.astype(BF16), ident=ident.astype(BF16))


def _run(in_maps, trace=False):
    from concourse.bass_utils import run_bass_kernel_spmd
    nc = _program()
    kwargs = {}
    if trace:
        kwargs = dict(trace=True, trace_cores=list(range(NCORES)))
    return run_bass_kernel_spmd(nc, in_maps, core_ids=list(range(NCORES)),
                                **kwargs)


def kernel(x, Wq, Wkv_down, Wk_up, Wv_up, Wo, _trace=False):
    x = np.ascontiguousarray(np.asarray(x, dtype=np.float32))
    Wq = np.ascontiguousarray(np.asarray(Wq, dtype=np.float32))
    Wkv_down = np.ascontiguousarray(np.asarray(Wkv_down, dtype=np.float32))
    Wk_up = np.ascontiguousarray(np.asarray(Wk_up, dtype=np.float32))
    Wv_up = np.ascontiguousarray(np.asarray(Wv_up, dtype=np.float32))
    Wo = np.ascontiguousarray(np.asarray(Wo, dtype=np.float32))

    cos, sin = _rope_tables()
    in_maps = []
    for c in range(NCORES):
        b, ch = divmod(c, 4)
        in_maps.append(_core_inputs(x, Wq, Wkv_down, Wk_up, Wv_up, Wo,
                                    cos, sin, b, ch))
    res = _run(in_maps, trace=_trace)
    out = np.empty((B, T, D), np.float32)
    for c in range(NCORES):
        b, ch = divmod(c, 4)
        out[b, ch * TQ:(ch + 1) * TQ, :] = res.results[c]["outT"].T
    if _trace:
        kernel.last_results = res
    return out


# revision 30
# speedup vs baseline: 1.0073x; 1.0073x over previous
"""Block-causal sparse attention (MLA latent KV + GQA + RoPE) on 8 TRN2 cores.

Sharding: 2 batches x 4 query-chunks of 512 tokens (T-sharding). Each core
computes its 512 output rows end-to-end (q/kv projections, sparse attention,
o-projection) over a gathered key set of 768 window rows + 32 global slots.
Everything runs in a transposed layout (feature dim on partitions) so no
fp32 DMA-transpose is ever needed: the host supplies x^T slices and the
kernel returns out^T, which the host transposes back (free in numpy).

All matmul operands are bf16 (fp32 PSUM accumulation). Attention (stage C2)
computes S^T = K^T q directly (k on partitions) so the exp itself evacuates
PSUM->SBUF and no P transposes are needed; P@V runs with q on the output
partitions using an ones-augmented V column, so softmax row-sums appear as
output column 128 and the reciprocal is a cheap [128,1] DVE op. Only the
small y tiles (4 per head) are PE-transposed back for the o-projection.
"""

import functools
import numpy as np
import ml_dtypes

BF16 = ml_dtypes.bfloat16

# Model constants (hardcoded per problem spec)
D = 2048        # d_model
L = 512         # MLA latent
NH = 16         # query heads
NKV = 4         # kv heads
HD = 128        # head dim
B, T = 2, 2048
BLOCK = 128
WINDOW = 256
GEV = 64        # global every
THETA = 10000.0

# Sharding geometry
NCORES = 8
TQ = 512        # queries per core
KWIN = 768      # window key rows (t0-256 .. t0+512)
NG = 32         # global slots (padded)
KT = 896        # key layout: 768 window | 32 global | 96 zero pad
KQ0 = 256       # query cols inside key layout
NEG = -1e30
SCALE = 1.0 / float(np.sqrt(HD))

# S^T bank packing: (bank, column offset) of each window block w; widths are
# the number of valid q columns (q range [max(0,w-2)*128, (min(3,w)+1)*128)).
W_BANK = {0: (0, 0), 2: (0, 128), 3: (1, 0), 5: (1, 384), 1: (2, 0), 4: (2, 256)}
W_QLO = {w: max(0, w - 2) for w in range(6)}
W_QHI = {w: min(3, w) for w in range(6)}


def _build_program(loop_n=None):
    import contextlib
    import concourse.bacc as bacc
    import concourse.tile as tile
    import concourse.mybir as mybir

    f32 = mybir.dt.float32
    bf = mybir.dt.bfloat16
    EXP = mybir.ActivationFunctionType.Exp
    CPY = mybir.ActivationFunctionType.Copy

    nc = bacc.Bacc("TRN2", target_bir_lowering=False, debug=False)

    xT = nc.dram_tensor("xT", [D, KT], bf, kind="ExternalInput")
    wq = nc.dram_tensor("wq", [D, NH * HD], bf, kind="ExternalInput")
    wkv = nc.dram_tensor("wkv", [D, L], bf, kind="ExternalInput")
    wku = nc.dram_tensor("wku", [L, NKV * HD], bf, kind="ExternalInput")
    wvu = nc.dram_tensor("wvu", [L, NKV * HD], bf, kind="ExternalInput")
    wo = nc.dram_tensor("wo", [NH * HD, D], bf, kind="ExternalInput")
    cosT = nc.dram_tensor("cosT", [HD, KT], f32, kind="ExternalInput")
    sinT = nc.dram_tensor("sinT", [HD, KT], f32, kind="ExternalInput")
    # bias zone tables (transposed layout: [k, q]):
    #  tblA [128, 2, 128]: (w0,l0) | (w2,l2)   -> bankA cols 0:128, 384:512
    #  tblB [128, 128]:    (w3,l3)             -> bankB cols 256:384
    #  tblC [128, 256]:    (w1,l0|l1)          -> bankC cols 0:256
    #  tblG [32, 512]:     globals, all l      -> glob bank
    tblA = nc.dram_tensor("tblA", [128, 2, 128], bf, kind="ExternalInput")
    tblB = nc.dram_tensor("tblB", [128, 128], bf, kind="ExternalInput")
    tblC = nc.dram_tensor("tblC", [128, 256], bf, kind="ExternalInput")
    tblG = nc.dram_tensor("tblG", [32, 512], bf, kind="ExternalInput")
    rotM = nc.dram_tensor("rotM", [HD, HD], bf, kind="ExternalInput")
    ident = nc.dram_tensor("ident", [HD, HD], bf, kind="ExternalInput")
    outT = nc.dram_tensor("outT", [D, TQ], f32, kind="ExternalOutput")

    def act_copy(out, in_, scale=1.0):
        nc.scalar.activation(out, in_, CPY, scale=scale)

    with tile.TileContext(nc) as tc, contextlib.ExitStack() as _es:
        if loop_n:
            _es.enter_context(tc.For_i(0, loop_n, 1))
        with tc.tile_pool(name="const", bufs=1) as constp:
            cos_sb = constp.tile([HD, KT], f32)
            sin_sb = constp.tile([HD, KT], f32)
            tA_sb = constp.tile([128, 2, 128], bf)
            tB_sb = constp.tile([128, 128], bf)
            tC_sb = constp.tile([128, 256], bf)
            tG_sb = constp.tile([32, 512], bf)
            rot_sb = constp.tile([HD, HD], bf)
            id_sb = constp.tile([HD, HD], bf)

            def emit_const_dmas():
                # emitted after stage A's first x/w slices so the first
                # matmuls aren't stuck behind ~2MB of constant loads
                nc.sync.dma_start(cos_sb[:], cosT[:])
                nc.sync.dma_start(sin_sb[:], sinT[:])
                nc.sync.dma_start(tA_sb[:], tblA[:])
                nc.sync.dma_start(tB_sb[:], tblB[:])
                nc.sync.dma_start(tC_sb[:], tblC[:])
                nc.sync.dma_start(tG_sb[:], tblG[:])
                nc.sync.dma_start(rot_sb[:], rotM[:])
                nc.sync.dma_start(id_sb[:], ident[:])

            xtq_sb = constp.tile([128, 16, TQ], bf)      # x^T query cols
            qT_all = constp.tile([HD, NH, TQ], bf)       # roped q^T per head
            kT_sb = constp.tile([HD, NKV, KT], bf)       # roped K^T per kv head
            # V rows x (w, kv, hd + ones col). w=6 holds global-slot V rows.
            V_aug = constp.tile([128, 7, NKV, HD + 1], bf)
            nc.vector.memset(V_aug[:, :, :, HD:HD + 1], 1.0)
            yT_sb = constp.tile([HD, NH, TQ], bf)        # attention out^T
            y_all = constp.tile([128, NH, 4, HD], bf)    # y [q, hd] per (h, l)
            pD_sb = constp.tile([128, 16, TQ], f32)      # stage-D hk<8 partials

            with tc.tile_pool(name="ckvp", bufs=1) as ckvp:
                ckv_sb = ckvp.tile([128, 4, KT], bf)
                wku_sb = ckvp.tile([128, 4, NKV * HD], bf)
                wvu_sb = ckvp.tile([128, 4, NKV * HD], bf)

                def emit_kvw_dmas():
                    for lk in range(4):
                        nc.sync.dma_start(wku_sb[:, lk, :],
                                          wku[lk * 128:(lk + 1) * 128, :])
                        nc.sync.dma_start(wvu_sb[:, lk, :],
                                          wvu[lk * 128:(lk + 1) * 128, :])

                # ---- Stage A: c_kv^T = Wkv_down^T @ x^T  -> [L=4x128, KT] ----
                with tc.tile_pool(name="wx", bufs=3) as wxp, \
                     tc.tile_pool(name="psA", bufs=1, space="PSUM") as psA:
                    ps_ckv = [psA.tile([128, KT], f32, tag=f"ckv{lt}", name=f"ckv{lt}")
                              for lt in range(4)]
                    for k in range(16):
                        xt_k = wxp.tile([128, KT], bf, tag="xt")
                        nc.sync.dma_start(xt_k[:], xT[k * 128:(k + 1) * 128, :])
                        wkv_k = wxp.tile([128, L], bf, tag="wkv")
                        nc.sync.dma_start(wkv_k[:], wkv[k * 128:(k + 1) * 128, :])
                        if k == 0:
                            emit_const_dmas()
                            emit_kvw_dmas()
                        # stash query cols for stage C1
                        nc.vector.tensor_copy(xtq_sb[:, k, :], xt_k[:, KQ0:KQ0 + TQ])
                        for lt in range(4):
                            for c0, c1 in ((0, 512), (512, KT)):
                                nc.tensor.matmul(
                                    ps_ckv[lt][:, c0:c1],
                                    wkv_k[:, lt * 128:(lt + 1) * 128],
                                    xt_k[:, c0:c1],
                                    start=(k == 0), stop=(k == 15),
                                )
                    for lt in range(4):
                        nc.vector.tensor_copy(ckv_sb[:, lt, :], ps_ckv[lt][:])

                # ---- Stage C1: q projection + RoPE for all heads ----
                with tc.tile_pool(name="wqp", bufs=3) as wqp, \
                     tc.tile_pool(name="tmpQ", bufs=2) as tmpQ, \
                     tc.tile_pool(name="psq", bufs=2, space="PSUM") as psq, \
                     tc.tile_pool(name="psr", bufs=2, space="PSUM") as psr:
                    for h in range(NH):
                        wq_h = wqp.tile([128, 16, HD], bf, tag="wqh")
                        nc.sync.dma_start(
                            wq_h[:],
                            wq[:, h * HD:(h + 1) * HD].rearrange(
                                "(ko p) m -> p ko m", p=128),
                        )
                        ps_q = psq.tile([128, TQ], f32, tag="q")
                        for k in range(16):
                            nc.tensor.matmul(
                                ps_q[:], wq_h[:, k, :], xtq_sb[:, k, :],
                                start=(k == 0), stop=(k == 15),
                            )
                        qh_r = tmpQ.tile([128, TQ], bf, tag="qhr")
                        act_copy(qh_r[:], ps_q[:])
                        t_qc = tmpQ.tile([128, TQ], f32, tag="tqc")
                        nc.vector.tensor_mul(t_qc[:], ps_q[:],
                                             cos_sb[:, KQ0:KQ0 + TQ])
                        ps_rt = psr.tile([128, TQ], f32, tag="qrot")
                        nc.tensor.matmul(ps_rt[:], rot_sb[:], qh_r[:],
                                         start=True, stop=True)
                        qt1 = tmpQ.tile([128, TQ], f32, tag="qt1")
                        nc.vector.tensor_mul(qt1[:], ps_rt[:],
                                             sin_sb[:, KQ0:KQ0 + TQ])
                        nc.vector.tensor_add(qT_all[:, h, :], t_qc[:], qt1[:])

                # ---- Stage B: K^T (roped) and V ----
                # V-up matmul groups are interleaved between the K-rope
                # groups: the rope chain (PE->ACT->PE->DVE) leaves the PE
                # idle long enough for the HAM clock gate to re-throttle
                # right before C2; the dense V matmuls fill those gaps.
                with tc.tile_pool(name="tmpB", bufs=2) as tmpB, \
                     tc.tile_pool(name="psB", bufs=1, space="PSUM") as psB:
                    def emit_v(tt):
                        ps_v = psB.tile([128, 512], f32, tag="v")
                        for lk in range(4):
                            nc.tensor.matmul(
                                ps_v[:],
                                ckv_sb[:, lk, tt * 128:(tt + 1) * 128],
                                wvu_sb[:, lk, :],
                                start=(lk == 0), stop=(lk == 3),
                            )
                        nc.vector.tensor_copy(
                            V_aug[:, tt, :, 0:HD],
                            ps_v[:].rearrange("p (g d) -> p g d", g=NKV))

                    v_sched = {0: (0, 0), 1: (0, 1), 2: (1, 3), 3: (3, 7)}
                    for g in range(NKV):
                        ps_kh = psB.tile([128, KT], f32, tag="kh")
                        for lk in range(4):
                            for c0, c1 in ((0, 512), (512, KT)):
                                nc.tensor.matmul(
                                    ps_kh[:, c0:c1],
                                    wku_sb[:, lk, g * 128:(g + 1) * 128],
                                    ckv_sb[:, lk, c0:c1],
                                    start=(lk == 0), stop=(lk == 3),
                                )
                        kh_r = tmpB.tile([128, KT], bf, tag="khr")
                        act_copy(kh_r[:], ps_kh[:])
                        t_kc = tmpB.tile([128, KT], f32, tag="tkc")
                        nc.vector.tensor_mul(t_kc[:], ps_kh[:], cos_sb[:])
                        ps_rot = psB.tile([128, KT], f32, tag="rot")
                        for c0, c1 in ((0, 512), (512, KT)):
                            nc.tensor.matmul(ps_rot[:, c0:c1], rot_sb[:],
                                             kh_r[:, c0:c1], start=True, stop=True)
                        t1 = tmpB.tile([128, KT], f32, tag="t1")
                        nc.vector.tensor_mul(t1[:], ps_rot[:], sin_sb[:])
                        nc.vector.tensor_add(kT_sb[:, g, :], t_kc[:], t1[:])
                        for tt in range(*v_sched[g]):
                            emit_v(tt)

            # ---- Stage C2: sparse attention in S^T layout ----
            # S^T = K^T q (k rows on partitions) -> bias-add -> exp (evacuates
            # PSUM->SBUF) -> P@V with q on output partitions (ones column of
            # V_aug gives softmax sums in col 128) -> recip+scale -> transpose.
            with tc.tile_pool(name="ptp", bufs=2) as ptp, \
                 tc.tile_pool(name="rcp", bufs=2) as rcp, \
                 tc.tile_pool(name="wo1p", bufs=2) as wo1p, \
                 tc.tile_pool(name="psS", bufs=1, space="PSUM") as psS, \
                 tc.tile_pool(name="psG", bufs=1, space="PSUM") as psG, \
                 tc.tile_pool(name="psY", bufs=1, space="PSUM") as psY, \
                 tc.tile_pool(name="psD1", bufs=1, space="PSUM") as psD1, \
                 tc.tile_pool(name="psT", bufs=1, space="PSUM") as psT:

                st = psS.tile([128, 3, 512], f32, name="st")
                stg = psG.tile([32, 512], f32, name="stg")
                ya = psY.tile([128, 2, HD + 1], f32, name="ya")
                yb = psY.tile([128, 2, HD + 1], f32, name="yb")
                pt_tiles = [None] * NH
                r_tiles = [None] * NH

                def st_pe_ops(h):
                    """PE ops of head h's S^T, with bias/exp emitted inline as
                    their input banks complete (w order: banks A, B, C, glob)."""
                    g = h // 4
                    ptw = ptp.tile([128, 3, 512], bf, tag="ptw",
                                   name=f"ptw{h % 2}")
                    ptg = ptp.tile([32, 512], bf, tag="ptg", name=f"ptg{h % 2}")
                    pt_tiles[h] = (ptw, ptg)

                    def mk_w(w, start=False):
                        def op():
                            b, off = W_BANK[w]
                            qlo, qhi = W_QLO[w], W_QHI[w]
                            nc.tensor.matmul(
                                st[:, b, off:off + (qhi - qlo + 1) * 128],
                                kT_sb[:, g, w * 128:(w + 1) * 128],
                                qT_all[:, h, qlo * 128:(qhi + 1) * 128],
                                start=start, stop=True,
                                skip_group_check=True,
                            )
                        return op

                    # Bias zones are seeded by identity-weight matmuls
                    # (start=True writes the bias table into PSUM and sets
                    # has_written only in the bias zones; the S^T matmuls then
                    # accumulate onto them, overwriting elsewhere). This keeps
                    # the per-head critical cycle st(h+1) -> exp(h) -> st(h)
                    # on two engines instead of three (no DVE hop).
                    # Bank C (w1, w4) is built and exp'd first so head h+1's
                    # first matmul only waits on this early exp.
                    def bias_mm(out_ap, tbl_ap, lhs, start=True):
                        # start=True on the first bias matmul of a bank clears
                        # the bank's stale has_written bits from the previous
                        # head; later seeds in the same bank use start=False
                        # (bits there are clear, so they overwrite).
                        nc.tensor.matmul(out_ap, lhs, tbl_ap,
                                         start=start, stop=True,
                                         skip_group_check=True)

                    def op_biases():
                        # all bias seeds back to back: one id stationary load
                        # serves four matmuls (same-weight streams pipeline at
                        # N cycles instead of paying fill+drain each)
                        bias_mm(st[:, 2, 0:256], tC_sb[:], id_sb[:])
                        bias_mm(st[:, 0, 0:128], tA_sb[:, 0, :], id_sb[:])
                        bias_mm(st[:, 0, 384:512], tA_sb[:, 1, :], id_sb[:],
                                start=False)
                        bias_mm(st[:, 1, 256:384], tB_sb[:], id_sb[:])
                        bias_mm(stg[:], tG_sb[:], id_sb[0:NG, 0:NG])

                    def op_bankC():
                        mk_w(4)()
                        nc.scalar.activation(ptw[:, 2, :], st[:, 2, :], EXP,
                                             scale=SCALE)

                    def op_bankB():
                        mk_w(5)()
                        nc.scalar.activation(ptw[:, 0:2, :], st[:, 0:2, :], EXP,
                                             scale=SCALE)

                    def op_glob():
                        nc.tensor.matmul(
                            stg[:], kT_sb[:, g, KWIN:KWIN + NG],
                            qT_all[:, h, :], start=False, stop=True,
                            skip_group_check=True,
                        )
                        nc.scalar.activation(ptg[:], stg[:], EXP, scale=SCALE)

                    return [op_biases, mk_w(1), op_bankC, mk_w(0),
                            mk_w(2), mk_w(3), op_bankB, op_glob]

                def pv_pe_ops(h):
                    """16 PV matmuls of head h (each one LDWEIGHTS + short MM;
                    interleaved with the next head's S^T streams to keep the
                    PE array busy so HAM stays warm)."""
                    g = h // 4
                    ptw, ptg = pt_tiles[h]
                    ops = []
                    for l in range(4):
                        yt = (ya, yb)[l // 2]
                        for i, w in enumerate((l, l + 1, l + 2)):
                            def op(l=l, w=w, i=i, yt=yt):
                                b, off = W_BANK[w]
                                zoff = off + (l - W_QLO[w]) * 128
                                nc.tensor.matmul(
                                    yt[:, l % 2, :],
                                    ptw[:, b, zoff:zoff + 128],
                                    V_aug[:, w, g, :],
                                    start=(i == 0), stop=False,
                                    skip_group_check=True,
                                )
                            ops.append(op)

                        def opg(l=l, yt=yt):
                            nc.tensor.matmul(
                                yt[:, l % 2, :],
                                ptg[:, l * 128:(l + 1) * 128],
                                V_aug[0:NG, 6, g, :],
                                start=False, stop=True, skip_group_check=True,
                            )
                        ops.append(opg)
                    return ops

                def pv_post(h):
                    ra = rcp.tile([128, 2, 1], f32, tag="ra", name=f"ra{h % 2}")
                    rb = rcp.tile([128, 2, 1], f32, tag="rb", name=f"rb{h % 2}")
                    r_tiles[h] = (ra, rb)
                    nc.vector.reciprocal(ra[:], ya[:, :, HD:HD + 1])
                    nc.vector.reciprocal(rb[:], yb[:, :, HD:HD + 1])
                    # normalize + evacuate y [q, hd] (alternate DVE/ACT)
                    for l in range(4):
                        yt = (ya, yb)[l // 2]
                        rt = (ra, rb)[l // 2]
                        if l == 1:
                            nc.scalar.activation(
                                y_all[:, h, l, :], yt[:, l % 2, 0:HD], CPY,
                                scale=rt[:, l % 2, :])
                        else:
                            nc.vector.tensor_scalar_mul(
                                y_all[:, h, l, :], yt[:, l % 2, 0:HD],
                                rt[:, l % 2, :])

                def transp(h):
                    ps_t = psT.tile([128, 512], bf, tag="pt", name="pst")
                    for l in range(4):
                        nc.tensor.transpose(
                            ps_t[:, l * 128:(l + 1) * 128], y_all[:, h, l, :],
                            id_sb[:])
                    if h % 2 == 0:
                        nc.vector.tensor_copy(yT_sb[:, h, :], ps_t[:])
                    else:
                        act_copy(yT_sb[:, h, :], ps_t[:])

                def d1_od(od, q):
                    # first half of the o-projection (hk 0..7, in quarters),
                    # interleaved into the head loop as soon as its yT inputs
                    # exist: the long N=512 streams fill C2's dependency
                    # stalls and keep the HAM clock gate warm on every core;
                    # partials land in SBUF for stage D2.
                    wo_od = wo1p.tile([128, 4, 128], bf, tag="wo1")
                    nc.sync.dma_start(
                        wo_od[:],
                        wo[q * 512:(q + 1) * 512,
                           od * 128:(od + 1) * 128].rearrange(
                            "(ko p) m -> p ko m", p=128),
                    )
                    ps_o = psD1.tile([128, TQ], f32, tag="d1", name="d1")
                    for hk in range(4):
                        nc.tensor.matmul(
                            ps_o[:], wo_od[:, hk, :], yT_sb[:, q * 4 + hk, :],
                            start=(hk == 0), stop=(hk == 3),
                        )
                    if q == 0:
                        nc.vector.tensor_copy(pD_sb[:, od, :], ps_o[:])
                    else:
                        nc.vector.tensor_add(pD_sb[:, od, :], ps_o[:],
                                             pD_sb[:, od, :])

                def interleave(st_ops, pv_ops):
                    # ~2 PV matmuls (LDW-heavy, short streams) per S^T op
                    # (long streams): the S^T streaming hides the PV weight
                    # loads so the PE array never idles long enough to cool.
                    si, pi = 0, 0
                    while si < len(st_ops) or pi < len(pv_ops):
                        if si < len(st_ops):
                            st_ops[si]()
                            si += 1
                        for _ in range(3):
                            if pi < len(pv_ops):
                                pv_ops[pi]()
                                pi += 1

                d1_sched = {5: (0, 0, 4), 6: (0, 4, 8), 7: (0, 8, 12),
                            8: (0, 12, 16), 9: (1, 0, 4), 10: (1, 4, 8),
                            11: (1, 8, 12), 12: (1, 12, 16)}
                for h in range(NH):
                    interleave(st_pe_ops(h), pv_pe_ops(h - 1) if h else [])
                    if h > 0:
                        pv_post(h - 1)
                        transp(h - 1)
                    if h in d1_sched:
                        q, o0, o1 = d1_sched[h]
                        for od in range(o0, o1):
                            d1_od(od, q)
                interleave([], pv_pe_ops(NH - 1))
                pv_post(NH - 1)
                transp(NH - 1)

            # ---- Stage D2: second half of o-projection (hk 8..15) + partials
            with tc.tile_pool(name="wop", bufs=3) as wop, \
                 tc.tile_pool(name="tmpD", bufs=3) as tmpD, \
                 tc.tile_pool(name="psD", bufs=2, space="PSUM") as psD:
                for od in range(16):
                    wo_od = wop.tile([128, 8, 128], bf, tag="wo")
                    nc.sync.dma_start(
                        wo_od[:],
                        wo[1024:2048, od * 128:(od + 1) * 128].rearrange(
                            "(ko p) m -> p ko m", p=128),
                    )
                    ps_o = psD.tile([128, TQ], f32, tag="o")
                    for hk in range(8):
                        nc.tensor.matmul(
                            ps_o[:], wo_od[:, hk, :], yT_sb[:, 8 + hk, :],
                            start=(hk == 0), stop=(hk == 7),
                        )
                    ob = tmpD.tile([128, TQ], f32, tag="ob")
                    nc.vector.tensor_add(ob[:], ps_o[:], pD_sb[:, od, :])
                    nc.sync.dma_start(outT[od * 128:(od + 1) * 128, :], ob[:])

    nc.finalize()
    return nc


@functools.lru_cache(maxsize=1)
def _program():
    return _build_program()


def _rope_tables():
    freqs = 1.0 / (THETA ** (np.arange(0, HD, 2, dtype=np.float32) / HD))
    emb = np.arange(T, dtype=np.float32)[:, None] * freqs[None, :]  # [T, 64]
    cos = np.concatenate([np.cos(emb), np.cos(emb)], axis=-1)  # [T, 128]
    sin = np.concatenate([np.sin(emb), np.sin(emb)], axis=-1)
    return cos.astype(np.float32), sin.astype(np.float32)


def _masked(qpos, kpos):
    """Reference sparsity rule. qpos [Q], kpos [K] -> bool [Q, K] (True=masked)."""
    qb = qpos[:, None] // BLOCK
    kb = kpos[None, :] // BLOCK
    future = kb > qb
    outside = np.abs(kpos[None, :] - qpos[:, None]) > WINDOW
    glob = (kpos[None, :] % GEV) == 0
    return (outside & ~glob) | future


def _zone_bias(kp, t0, w, l):
    """Bias [128 k, 128 q] for window block w vs query block l (S^T layout)."""
    kpos = kp[w * 128:(w + 1) * 128]
    qpos = t0 + l * 128 + np.arange(128)
    m = np.where(kpos[:, None] < 0, True,
                 _masked(qpos, np.maximum(kpos, 0)).T)
    return np.where(m, NEG, 0.0).astype(np.float32)


def _core_inputs(x, Wq, Wkv, Wku, Wvu, Wo, cos, sin, b, ch):
    t0 = ch * TQ
    kp = np.full(KT, -1, dtype=np.int64)
    kp[0:KWIN] = np.arange(t0 - WINDOW, t0 + TQ)
    globpos = np.arange(0, max(0, t0 - WINDOW + 3 * 128), GEV)
    assert len(globpos) <= NG
    kp[KWIN:KWIN + len(globpos)] = globpos
    valid = kp >= 0

    xT = np.zeros((D, KT), BF16)
    xT[:, valid] = x[b, kp[valid]].T.astype(BF16)
    cosT = np.zeros((HD, KT), np.float32)
    sinT = np.zeros((HD, KT), np.float32)
    cosT[:, valid] = cos[kp[valid]].T
    sinT[:, valid] = sin[kp[valid]].T

    # Window-zone bias tables (S^T layout: [k rows, q cols]).
    tblA = np.stack([_zone_bias(kp, t0, 0, 0), _zone_bias(kp, t0, 2, 2)], 0)
    tblA = tblA.transpose(1, 0, 2)                     # [128, 2, 128]
    tblB = _zone_bias(kp, t0, 3, 3)                    # [128, 128]
    tblC = np.concatenate([_zone_bias(kp, t0, 1, 0),
                           _zone_bias(kp, t0, 1, 1)], axis=1)  # [128, 256]
    # sanity: every other (w, l) zone must be mask-free
    for w in range(6):
        for l in range(W_QLO[w], W_QHI[w] + 1):
            if (w, l) in ((0, 0), (2, 2), (3, 3), (1, 0), (1, 1)):
                continue
            assert not (_zone_bias(kp, t0, w, l) < 0).any(), (w, l)

    # Globals bias [32 slots, 512 q]: validity + sparsity + in-window dedup.
    kg = kp[KWIN:KWIN + NG]
    qpos = t0 + np.arange(TQ)
    mg = np.where(kg[:, None] < 0, True,
                  _masked(qpos, np.maximum(kg, 0)).T)
    lblk = np.arange(TQ) // 128
    mg = mg | (kg[:, None] >= (t0 - WINDOW + lblk[None, :] * 128))
    tblG = np.where(mg, NEG, 0.0).astype(np.float32)

    rotM = np.zeros((HD, HD), np.float32)
    rotM[np.arange(64), np.arange(64) + 64] = 1.0
    rotM[np.arange(64) + 64, np.arange(64)] = -1.0
    ident = np.eye(HD, dtype=np.float32)

    return dict(xT=xT,
                wq=Wq.astype(BF16), wkv=Wkv.astype(BF16),
                wku=Wku.astype(BF16), wvu=Wvu.astype(BF16),
                wo=Wo.astype(BF16),
                cosT=cosT, sinT=sinT,
                tblA=tblA.astype(BF16), tblB=tblB.astype(BF16),
                tblC=tblC.astype(BF16), tblG=tblG.astype(BF16),
                rotM=rotM.astype(BF16), ident=ident.astype(BF16))


def _run(in_maps, trace=False):
    from concourse.bass_utils import run_bass_kernel_spmd
    nc = _program()
    kwargs = {}
    if trace:
        kwargs = dict(trace=True, trace_cores=list(range(NCORES)))
    return run_bass_kernel_spmd(nc, in_maps, core_ids=list(range(NCORES)),
                                **kwargs)


def kernel(x, Wq, Wkv_down, Wk_up, Wv_up, Wo, _trace=False):
    x = np.ascontiguousarray(np.asarray(x, dtype=np.float32))
    Wq = np.ascontiguousarray(np.asarray(Wq, dtype=np.float32))
    Wkv_down = np.ascontiguousarray(np.asarray(Wkv_down, dtype=np.float32))
    Wk_up = np.ascontiguousarray(np.asarray(Wk_up, dtype=np.float32))
    Wv_up = np.ascontiguousarray(np.asarray(Wv_up, dtype=np.float32))
    Wo = np.ascontiguousarray(np.asarray(Wo, dtype=np.float32))

    cos, sin = _rope_tables()
    in_maps = []
    for c in range(NCORES):
        b, ch = divmod(c, 4)
        in_maps.append(_core_inputs(x, Wq, Wkv_down, Wk_up, Wv_up, Wo,
                                    cos, sin, b, ch))
    res = _run(in_maps, trace=_trace)
    out = np.empty((B, T, D), np.float32)
    for c in range(NCORES):
        b, ch = divmod(c, 4)
        out[b, ch * TQ:(ch + 1) * TQ, :] = res.results[c]["outT"].T
    if _trace:
        kernel.last_results = res
    return out
